# revision 1
# baseline (speedup 1.0000x reference)
"""RandomProjectionQuantizer Bass kernel for Trainium2 (8 NeuronCores).

labels[b, l] = argmin_c( ||cb[:,c]||^2 - 2 * (x[b,l] @ W.T) . cb[:,c] )

Exact-fp32-equivalent math on the PE via hi/lo FP22 compensation:
  x = xh + xl, W = Wh + Wl, t = x@W.T ~= xh@Wh + xh@Wl + xl@Wh   (drop lo*lo)
  s = t@cb    ~= th@cbh + th@cbl + tl@cbh
All products are exact in the PE's FP22 multiply path (float32r dtype at
1 cycle/row vs true fp32's 4), accumulation is fp32 in PSUM.
Argmin is a single-pass custom DVE op: running-min scan + index encode,
streamed over the c-reversed scores so ties break to the first index,
exactly matching np.argmin.

Sharding: data-parallel over B (8 batches -> 8 cores), W/codebook
replicated. No cross-core communication.
"""

import numpy as np

import concourse.bacc as bacc
import concourse.mybir as mybir
from concourse import tile
from concourse.bass_utils import run_bass_kernel_spmd
from concourse.dve_spec import (Spec, Src0, Src1, C0, C1, Zero, MaxNeg,
                                AluOp, Idx, eq, select, scan, lower)
from concourse.dve_uop import DveOpSpec
from concourse import dve_ops as DOPS

B, L, D, Q, C = 8, 2048, 1024, 256, 4096
N_CORES = 8
TOK_BLOCK = 512          # tokens per pipeline block
N_BLOCKS = L // TOK_BLOCK
CBLK = 512               # c columns per matmul / psum bank
N_CBLK = C // CBLK
MASK_HI = np.uint32(0xFFFFF000)  # keep 12 significant bits (e10m11 FP22-exact)

f32 = mybir.dt.float32
f32r = mybir.dt.float32r
bf16 = mybir.dt.bfloat16


def _make_argmin_op():
    """Single-pass argmin over the free dim, streamed reversed.

    in0 = scores_raw (reversed over c), in1 = cb_sq (reversed, bcast to all
    partitions). s = in0 + in1. Positions where s equals its running min are
    prefix minima; encoding them as (C-1 - Idx) = forward index and taking
    accum MIN returns the first-occurrence forward argmin.
    """
    s = Src0 + Src1
    r = scan(AluOp.MIN, s, init=C0)
    body = select(eq(s, r), C1 - Idx, Zero - MaxNeg)

    def ref(in0, in1, c0, c1, c2):
        sv = (in0 + np.broadcast_to(in1, in0.shape)).astype(np.float32)
        rv = np.minimum.accumulate(sv, axis=-1)
        idx = np.arange(sv.shape[-1], dtype=np.float32)
        f = np.where(sv == rv, np.float32(c1) - idx, np.float32(3.4e38))
        acc = np.minimum(np.float32(c0), f.min(axis=-1, keepdims=True))
        return f.astype(np.float32), acc

    spec = Spec(body=body, accum=AluOp.MIN, accum_init=C0, reference=ref)
    name = "ARGMIN_REV_ANT"
    if name in DOPS._SUB_OPCODE_FOR_NAME:
        for op in DOPS.OPS:
            if op.name == name:
                return op
    row = DOPS._CUSTOM_DVE_ROW_BASE + len(DOPS.OPS)
    shas = {}
    for ver in ("v3", "v4"):
        d = DveOpSpec(name=name, opcode=row, uops=lower(spec, ver=ver), rd1_en=True)
        shas[ver] = d.sha(ver)
    op = DOPS.DveOp(name, spec, subdim=False, uops_sha=shas)
    DOPS.OPS.append(op)
    DOPS.CUSTOM_DVE_SPECS[name] = spec
    DOPS._SUB_OPCODE_FOR_NAME[name] = row
    return op


ARGMIN_OP = _make_argmin_op()


def build_kernel(repeats=1):
    """One-core program: 2048 tokens, full codebook. SPMD over 8 cores.

    repeats>1 re-runs the whole pipeline (for overhead-free timing via
    work-scaling); labels are simply overwritten each repeat."""
    nc = bacc.Bacc(None, target_bir_lowering=False)

    x_d = nc.dram_tensor("x", [L, D], f32, kind="ExternalInput")
    # W.T hi/lo packed host-side as [128, KD*Q] so each loads in ONE DMA
    wth_d = nc.dram_tensor("wth", [128, D // 128 * Q], f32r, kind="ExternalInput")
    wtl_d = nc.dram_tensor("wtl", [128, D // 128 * Q], f32r, kind="ExternalInput")
    cbh_d = nc.dram_tensor("cbh", [Q, C], f32r, kind="ExternalInput")
    cbl_d = nc.dram_tensor("cbl", [Q, C], f32r, kind="ExternalInput")
    cbsq_d = nc.dram_tensor("cbsqr", [1, C], f32, kind="ExternalInput")  # reversed
    id_d = nc.dram_tensor("ident", [128, 128], f32, kind="ExternalInput")
    lab_d = nc.dram_tensor("labels", [L // 128, 128], f32, kind="ExternalOutput")

    KD = D // 128   # 8 d-chunks
    KQ = Q // 128   # 2 q-chunks

    with tile.TileContext(nc) as tc:
        with (
            tc.tile_pool(name="const", bufs=1) as constp,
            tc.tile_pool(name="cb", bufs=1) as cbp,
            tc.tile_pool(name="stage", bufs=1) as stagep,
            tc.tile_pool(name="xt", bufs=1) as xtp,
            tc.tile_pool(name="tt", bufs=1) as ttp,
            tc.tile_pool(name="sc", bufs=2) as scp,
            tc.tile_pool(name="misc", bufs=1) as miscp,
            tc.tile_pool(name="ps_tr", bufs=2, space="PSUM") as ps_tr,
            tc.tile_pool(name="ps_tt", bufs=2, space="PSUM") as ps_tt,
            tc.tile_pool(name="ps_sc", bufs=4, space="PSUM") as ps_sc,
        ):
            ident = constp.tile([128, 128], f32)
            nc.sync.dma_start(ident[:], id_d[:])
            # Constants go on the SWDGE (gpsimd) queue so the token-stage
            # DMAs on the HWDGE (sync) queue aren't stuck behind ~12MB of
            # codebook — the first transpose can start within ~3us.
            wth_sb = constp.tile([128, KD * Q], f32r, name="wth_sb")
            wtl_sb = constp.tile([128, KD * Q], f32r, name="wtl_sb")
            for k in range(KD):
                nc.gpsimd.dma_start(wth_sb[:, k * Q:(k + 1) * Q], wth_d[:, k * Q:(k + 1) * Q])
                nc.gpsimd.dma_start(wtl_sb[:, k * Q:(k + 1) * Q], wtl_d[:, k * Q:(k + 1) * Q])
            wth = [wth_sb[:, k * Q:(k + 1) * Q] for k in range(KD)]
            wtl = [wtl_sb[:, k * Q:(k + 1) * Q] for k in range(KD)]
            cbh = [cbp.tile([128, C], f32r, tag=f"cbh{q}", name=f"cbh{q}") for q in range(KQ)]
            cbl = [cbp.tile([128, C], f32r, tag=f"cbl{q}", name=f"cbl{q}") for q in range(KQ)]
            # Load the four cb tiles half-C at a time, interleaved, so the
            # first score matmuls (which touch all four tiles but only low
            # c-blocks) start after ~4MB instead of the full 8.4MB.
            for chalf in range(2):
                c0 = chalf * (C // 2)
                for q in range(KQ):
                    nc.gpsimd.dma_start(cbh[q][:, c0:c0 + C // 2],
                                        cbh_d[q * 128:(q + 1) * 128, c0:c0 + C // 2])
                    nc.gpsimd.dma_start(cbl[q][:, c0:c0 + C // 2],
                                        cbl_d[q * 128:(q + 1) * 128, c0:c0 + C // 2])
            cbsq = constp.tile([128, C], f32)
            nc.gpsimd.dma_start(cbsq[:], cbsq_d[0].partition_broadcast(128))

            labels_sb = miscp.tile([128, L // 128], f32)
            dump = miscp.tile([128, C], bf16)

            for rep in range(repeats):
              for blk in range(N_BLOCKS):
                t0 = blk * TOK_BLOCK
                # ---- transpose x (hi/lo) on PE -> xT [d, tok] tiles.
                # Stage half the D dim at a time so the 4 token sub-tiles are
                # live together (distinct tags) without blowing SBUF.
                xth = [xtp.tile([128, TOK_BLOCK], f32r, tag=f"xth{k}", name=f"xth{blk}_{k}") for k in range(KD)]
                xtl = [xtp.tile([128, TOK_BLOCK], f32r, tag=f"xtl{k}", name=f"xtl{blk}_{k}") for k in range(KD)]
                for half in range(2):
                    d0 = half * 512
                    stg = [stagep.tile([128, 512], f32, tag=f"sg{s}", name=f"sg{blk}_{half}_{s}") for s in range(4)]
                    for s in range(4):
                        r0 = t0 + s * 128
                        nc.sync.dma_start(stg[s][:], x_d[r0:r0 + 128, d0:d0 + 512])
                    for k4 in range(4):
                        k = half * 4 + k4
                        pt = ps_tr.tile([128, TOK_BLOCK], f32, tag="ptr", name=f"pt{blk}_{k}")
                        for s in range(4):
                            nc.tensor.transpose(pt[:, s * 128:(s + 1) * 128],
                                                stg[s][:, k4 * 128:(k4 + 1) * 128], ident[:])
                        # exact on-chip hi/lo split: xth = rne22(xT) via the
                        # f32r-rounding ACT copy; xtl = xT - xth (FP22-exact).
                        xtf = stagep.tile([128, TOK_BLOCK], f32, tag="xtf", name=f"xtf{blk}_{k}", bufs=2)
                        nc.scalar.mul(xtf[:], pt[:], 1.0)
                        nc.scalar.mul(xth[k][:], pt[:], 1.0)
                        nc.vector.tensor_tensor(
                            out=xtl[k][:], in0=xtf[:],
                            in1=xth[k][:].bitcast(f32), op=mybir.AluOpType.subtract)

                # ---- mm1: tT[q, tok] = sum_d W.T[d,q].T @ xT[d,tok] (3-term)
                tth = [ttp.tile([128, TOK_BLOCK], f32r, tag=f"tth{q}", name=f"tth{blk}_{q}") for q in range(KQ)]
                ttl = [ttp.tile([128, TOK_BLOCK], f32r, tag=f"ttl{q}", name=f"ttl{blk}_{q}") for q in range(KQ)]
                for q in range(KQ):
                    pt = ps_tt.tile([128, TOK_BLOCK], f32, tag="ptt", name=f"ptt{blk}_{q}")
                    first = True
                    for k in range(KD):
                        wh = wth[k][:, q * 128:(q + 1) * 128]
                        wl = wtl[k][:, q * 128:(q + 1) * 128]
                        nc.tensor.matmul(pt[:], wh, xth[k][:], start=first, stop=False)
                        first = False
                        nc.tensor.matmul(pt[:], wl, xth[k][:], start=False, stop=False)
                        nc.tensor.matmul(pt[:], wh, xtl[k][:], start=False,
                                         stop=(k == KD - 1))
                    # tT = -2 * t (exact scale), split hi/lo: tth = rne22(tT)
                    # (f32r write rounds to FP22), ttl = tT - tth (<=11 sig
                    # bits, FP22-exact).
                    tt_f = ttp.tile([128, TOK_BLOCK], f32, tag=f"ttf{q}", name=f"ttf{blk}_{q}")
                    nc.scalar.mul(tt_f[:], pt[:], -2.0)
                    nc.scalar.mul(tth[q][:], pt[:], -2.0)
                    nc.vector.tensor_tensor(
                        out=ttl[q][:], in0=tt_f[:],
                        in1=tth[q][:].bitcast(f32), op=mybir.AluOpType.subtract)

                # ---- mm2 + argmin per 128-token tile
                for j in range(4):
                    jj = blk * 4 + j
                    sc = scp.tile([128, C], f32, tag="scores", name=f"sc{jj}")
                    for b in range(N_CBLK):
                        ps = ps_sc.tile([128, CBLK], f32, tag="psc", name=f"psc{jj}_{b}")
                        first = True
                        for q in range(KQ):
                            th = tth[q][:, j * 128:(j + 1) * 128]
                            tl = ttl[q][:, j * 128:(j + 1) * 128]
                            rh = cbh[q][:, b * CBLK:(b + 1) * CBLK]
                            rl = cbl[q][:, b * CBLK:(b + 1) * CBLK]
                            nc.tensor.matmul(ps[:], th, rh, start=first, stop=False)
                            first = False
                            nc.tensor.matmul(ps[:], th, rl, start=False, stop=False)
                            nc.tensor.matmul(ps[:], tl, rh, start=False,
                                             stop=(q == KQ - 1))
                        # write c-block REVERSED into the scores tile
                        dst = sc[:, C - (b + 1) * CBLK: C - b * CBLK][:, ::-1]
                        nc.scalar.mul(dst, ps[:], 1.0)
                    nc.vector._custom_dve(
                        ARGMIN_OP, out=dump[:], in0=sc[:], in1=cbsq[:],
                        s0=3.4e38, s1=float(C - 1),
                        accum_out=labels_sb[:, jj:jj + 1])

            nc.sync.dma_start(lab_d.rearrange("t p -> p t"), labels_sb[:])

    nc.compile()
    return nc


_NC_CACHE = None


def _get_nc():
    global _NC_CACHE
    if _NC_CACHE is None:
        _NC_CACHE = build_kernel()
    return _NC_CACHE


def _split_hi_lo(a):
    a = np.ascontiguousarray(a, np.float32)
    hi = (a.view(np.uint32) & MASK_HI).view(np.float32)
    lo = a - hi
    return hi, lo


def prepare_in_maps(input_values, W, codebook):
    x = np.ascontiguousarray(np.asarray(input_values), np.float32)
    W = np.ascontiguousarray(np.asarray(W), np.float32)
    cb = np.ascontiguousarray(np.asarray(codebook), np.float32)

    wth, wtl = _split_hi_lo(W.T)          # [D, Q]
    # pack [D, Q] -> [128, KD*Q]: column block k holds rows 128k..128k+128
    wth = np.ascontiguousarray(wth.reshape(D // 128, 128, Q).transpose(1, 0, 2).reshape(128, -1))
    wtl = np.ascontiguousarray(wtl.reshape(D // 128, 128, Q).transpose(1, 0, 2).reshape(128, -1))
    cbh, cbl = _split_hi_lo(cb)           # [Q, C]
    cb_sq = (cb.astype(np.float64) ** 2).sum(0).astype(np.float32)  # [C]
    cbsq_rev = np.ascontiguousarray(cb_sq[::-1], np.float32).reshape(1, C)
    ident = np.eye(128, dtype=np.float32)

    shared = {"wth": wth, "wtl": wtl, "cbh": cbh, "cbl": cbl,
              "cbsqr": cbsq_rev, "ident": ident}
    in_maps = []
    for b in range(N_CORES):
        in_maps.append({"x": np.ascontiguousarray(x[b]), **shared})
    return in_maps


def kernel(input_values, mask_time_indices=None, W=None, codebook=None,
           _trace=False):
    nc = _get_nc()
    in_maps = prepare_in_maps(input_values, W, codebook)
    res = run_bass_kernel_spmd(nc, in_maps, list(range(N_CORES)), trace=_trace)
    labels = np.stack([res.results[b]["labels"].ravel() for b in range(N_CORES)])
    out = labels.astype(np.int32)
    if _trace:
        kernel.last_exec_time_ns = res.exec_time_ns
        kernel.last_results = res
    return out

